# revision 19
# baseline (speedup 1.0000x reference)
"""Self-contained Trainium2 (Bass/Tile) DeformConv2d kernel.

kernel(x, offset, weight) -> np.ndarray [B, Cout, H, W] float32.
Data-parallel over batch: one SPMD Bass program per NeuronCore (8 cores).

Per core (one image): the bf16 x^T table lives in DRAM and is gathered with
4 int16 indices per (tap, pixel) — one per bilinear corner — laid out so the
4 corners of 32 pixels fill the 128 gather partitions (partition = 32*n + q).
DVE computes the 4 bilinear corner weights on-chip (L128 layout), 16 small
SBUF->SBUF DMAs shuffle them into the corner-stacked layout, and one narrow
[128,32] TensorScalarPtr per (tap, 32-px group) builds M = diag(w) @ K.
A single PE matmul per (tap, group, c-half) then performs
scale+combine+transpose+sum in one shot: valT[c,j] = sum_p g[p,c] * M[p,j].
The per-tap GEMM accumulates out[o, px] over taps in PSUM.
"""
import sys
import numpy as np

for _p in ("/opt/trn_rl_repo",):
    if _p not in sys.path:
        sys.path.insert(0, _p)

import concourse.bass as bass
import concourse.mybir as mybir
import concourse.tile as tile
from concourse import bacc
from concourse.bass_utils import run_bass_kernel_spmd

try:
    from ml_dtypes import bfloat16 as np_bf16
except ImportError:  # jax ships ml_dtypes; fall back via jax.numpy
    import jax.numpy as _jnp
    np_bf16 = _jnp.bfloat16

f32 = mybir.dt.float32
bf16 = mybir.dt.bfloat16
i32 = mybir.dt.int32
i16 = mybir.dt.int16
Alu = mybir.AluOpType
P = 128


def build_dcn(C=256, Cout=256, H=64, W=64, NT=9, NSTRIPE=8):
    HW = H * W
    S = HW // P              # 32 (L128 free index; s = pixel // 128)
    CB = C // P              # 2
    MB = Cout // P           # 2
    SPX = HW // NSTRIPE      # 512 pixels per stripe
    NSL = SPX // P           # 4 s_locals per stripe
    NG = SPX // 32           # 16 groups of 32 px per stripe
    NIDX = 4 * SPX           # 2048 gather slots per (tap, stripe)
    SW = NIDX // 16          # 128 idx free slots (16-wrap)
    FBIAS = 4.0 * max(H, W)
    FADD = FBIAS - 0.5       # HW f32->int cast is round-nearest-even

    nc = bacc.Bacc("TRN2", target_bir_lowering=False, debug=False)

    xtab = nc.declare_dram_parameter("xtab", [HW, C], bf16, isOutput=False)
    idxp = nc.declare_dram_parameter("idxp", [NSTRIPE, P, NT, SW], i16,
                                     isOutput=False)
    offs = nc.declare_dram_parameter("offs", [P, 4, NT, S], f32, isOutput=False)
    wt = nc.declare_dram_parameter("wt", [P, NT, CB, Cout], bf16, isOutput=False)
    kmat = nc.declare_dram_parameter("kmat", [P, 32], bf16, isOutput=False)
    perm = nc.declare_dram_parameter("perm", [P, 4, 4, P], bf16, isOutput=False)
    out = nc.declare_dram_parameter("out", [Cout, HW], bf16, isOutput=True)

    xtab_ap = bass.AP(xtab[:].tensor, 0, [[C, HW], [1, C]])

    with tile.TileContext(nc) as tc:
        with tc.tile_pool(name="persist", bufs=1) as pp:
            idx_sb = pp.tile([P, NSTRIPE, NT, SW], i16, name="idx_sb")
            offs_sb = pp.tile([P, 4, NT, S], f32, name="offs_sb")
            # offsets first (they gate the serial phase-1 chain), then
            # stripe-0 indices so gathers start immediately after
            nc.sync.dma_start(out=offs_sb[:], in_=offs[:])
            nc.sync.dma_start(out=idx_sb[:, 0], in_=idxp[0])
            oy, ox, by, bx = (offs_sb[:, i] for i in range(4))
            perm_sb = pp.tile([P, 4, 4, P], bf16, name="perm_sb")
            nc.sync.dma_start(out=perm_sb[:], in_=perm[:])
            for st in range(1, NSTRIPE):
                nc.sync.dma_start(out=idx_sb[:, st], in_=idxp[st])
            wtb = pp.tile([P, NT, CB, Cout], bf16, name="wtb")
            nc.sync.dma_start(out=wtb[:], in_=wt[:])
            ksb = pp.tile([P, 32], bf16, name="ksb")
            nc.sync.dma_start(out=ksb[:], in_=kmat[:])
            # corner weights in corner-stacked layout: [p=32n+q, d, k, s]
            wcol = pp.tile([P, 4, NT, S], f32, name="wcol")

            with (
                tc.tile_pool(name="gather", bufs=12) as g_pool,
                tc.tile_pool(name="mtiles", bufs=8) as m_pool,
                tc.tile_pool(name="vout", bufs=3) as v_pool,
                tc.tile_pool(name="obuf", bufs=2) as o_pool,
                tc.tile_pool(name="psum_out", bufs=1, space="PSUM") as pso_pool,
                tc.tile_pool(name="psum_val", bufs=3, space="PSUM") as psv_pool,
            ):
                # ---- phase 1: bilinear corner weights (L128: px = 128s+p)
                with tc.tile_pool(name="ph1", bufs=1) as sp:
                    names = ["py", "px", "y0", "x0", "ly", "lx",
                             "my0", "my1", "mx0", "mx1",
                             "vy0", "vy1", "ux0", "ux1",
                             "w0", "w1", "w2", "w3", "sa", "sb"]
                    T = {nm: sp.tile([P, NT, S], f32, name=nm) for nm in names}
                    vi = sp.tile([P, NT, S], i32, name="vi")

                    def tt(o, a, b, op):
                        nc.vector.tensor_tensor(out=T[o][:], in0=T[a][:],
                                                in1=T[b][:], op=op)

                    def ts(o, a, s1, op0, s2=None, op1=None):
                        if s2 is None:
                            nc.vector.tensor_scalar(
                                out=T[o][:], in0=T[a][:], scalar1=float(s1),
                                scalar2=None, op0=op0,
                            )
                        else:
                            nc.vector.tensor_scalar(
                                out=T[o][:], in0=T[a][:], scalar1=float(s1),
                                scalar2=float(s2), op0=op0, op1=op1,
                            )

                    nc.vector.tensor_tensor(out=T["py"][:], in0=oy,
                                            in1=by, op=Alu.add)
                    nc.vector.tensor_tensor(out=T["px"][:], in0=ox,
                                            in1=bx, op=Alu.add)

                    def floor_(dst, src):
                        ts("sa", src, FADD, Alu.add)
                        nc.vector.tensor_copy(out=vi[:], in_=T["sa"][:])
                        nc.vector.tensor_copy(out=T["sb"][:], in_=vi[:])
                        ts(dst, "sb", FBIAS, Alu.subtract)

                    def rng_mask(dst, v, lo, hi):
                        # mask = (clamp(v, lo, hi) == v)
                        ts("sa", v, lo, Alu.max, hi, Alu.min)
                        tt(dst, "sa", v, Alu.is_equal)

                    floor_("y0", "py")
                    tt("ly", "py", "y0", Alu.subtract)
                    rng_mask("my0", "y0", 0.0, H - 1)
                    rng_mask("my1", "y0", -1.0, H - 2)
                    floor_("x0", "px")
                    tt("lx", "px", "x0", Alu.subtract)
                    rng_mask("mx0", "x0", 0.0, W - 1)
                    rng_mask("mx1", "x0", -1.0, W - 2)

                    ts("sa", "ly", -1.0, Alu.mult, 1.0, Alu.add)
                    tt("vy0", "sa", "my0", Alu.mult)
                    tt("vy1", "ly", "my1", Alu.mult)
                    ts("sb", "lx", -1.0, Alu.mult, 1.0, Alu.add)
                    tt("ux0", "sb", "mx0", Alu.mult)
                    tt("ux1", "lx", "mx1", Alu.mult)

                    tt("w0", "vy0", "ux0", Alu.mult)
                    tt("w1", "vy0", "ux1", Alu.mult)
                    tt("w2", "vy1", "ux0", Alu.mult)
                    tt("w3", "vy1", "ux1", Alu.mult)
                    w4 = [T["w0"], T["w1"], T["w2"], T["w3"]]

                    # shuffle into corner-stacked layout on the PE:
                    # wcol[32n+q, d, k, s] = w_n[32d+q, k, s] via 16 constant
                    # permutation matmuls (PSUM-accumulated over n per d)
                    w4b = sp.tile([P, 4, NT, S], bf16, name="w4b")
                    for n in range(4):
                        nc.vector.tensor_copy(out=w4b[:, n], in_=w4[n][:])
                    for d in range(4):
                        # reuse the out_ps bank slot (WAR-serialized, pre-loop)
                        wps = pso_pool.tile([P, 512], f32, space="PSUM",
                                            name="ops0")
                        for n in range(4):
                            nc.tensor.matmul(
                                out=wps[:, :NT * S],
                                lhsT=perm_sb[:, n, d, :],
                                rhs=w4b[:, n].rearrange("p k s -> p (k s)"),
                                start=(n == 0), stop=(n == 3),
                            )
                        nc.scalar.copy(
                            out=wcol[:, d].rearrange("p k s -> p (k s)"),
                            in_=wps[:, :NT * S])

                # ---- main loop
                for st in range(NSTRIPE):
                    out_ps = [
                        pso_pool.tile([P, SPX], f32, space="PSUM", name=f"ops{m}")
                        for m in range(MB)
                    ]
                    for k in range(NT):
                        # HW SWDGE limit: 1024 indices per gather instruction;
                        # separate half tiles so combines start per half
                        nh = NIDX // 2
                        ghalf = []
                        for h in range(2):
                            gb = g_pool.tile([P, NG // 2, C], bf16, name=f"gb{h}")
                            nc.gpsimd.dma_gather(
                                gb[:], xtab_ap,
                                idx_sb[:, st, k, h * (SW // 2):(h + 1) * (SW // 2)],
                                nh, nh, C,
                            )
                            ghalf.append(gb)
                        mall = m_pool.tile([P, NG, 32], bf16, name="mall")
                        for sl in range(NSL):
                            for d in range(4):
                                g = 4 * sl + d
                                s_g = NSL * st + sl
                                nc.vector.tensor_scalar(
                                    out=mall[:, g, :], in0=ksb[:],
                                    scalar1=wcol[:, d, k, s_g:s_g + 1],
                                    scalar2=None, op0=Alu.mult,
                                )
                        val_ps = [
                            psv_pool.tile([P, SPX], f32, space="PSUM",
                                          name=f"vps{cb}")
                            for cb in range(CB)
                        ]
                        for sl in range(NSL):
                            for d in range(4):
                                g = 4 * sl + d
                                col = sl * P + 32 * d
                                gb = ghalf[g // (NG // 2)]
                                gg = g % (NG // 2)
                                for cb in range(CB):
                                    nc.tensor.matmul(
                                        out=val_ps[cb][:, col:col + 32],
                                        lhsT=gb[:, gg, cb * P:(cb + 1) * P],
                                        rhs=mall[:, g, :],
                                        start=True, stop=True,
                                    )
                        vsb0 = v_pool.tile([P, SPX], bf16, name="vsb0")
                        vsb1 = v_pool.tile([P, SPX], bf16, name="vsb1")
                        nc.vector.tensor_copy(out=vsb0[:], in_=val_ps[0][:])
                        nc.scalar.copy(out=vsb1[:], in_=val_ps[1][:])
                        for cb, vsb in ((0, vsb0), (1, vsb1)):
                            for mb in range(MB):
                                nc.tensor.matmul(
                                    out=out_ps[mb][:],
                                    lhsT=wtb[:, k, cb, mb * P:(mb + 1) * P],
                                    rhs=vsb[:],
                                    start=(k == 0 and cb == 0),
                                    stop=(k == NT - 1 and cb == CB - 1),
                                )
                    for mb in range(MB):
                        ob = o_pool.tile([P, SPX], bf16, name=f"ob{mb}")
                        if mb == 0:
                            nc.scalar.copy(out=ob[:], in_=out_ps[mb][:])
                        else:
                            nc.vector.tensor_copy(out=ob[:], in_=out_ps[mb][:])
                        nc.sync.dma_start(
                            out=out[mb * P:(mb + 1) * P, st * SPX:(st + 1) * SPX],
                            in_=ob[:],
                        )

    nc.compile()
    return nc


def host_prep(x_b, offset_b, weight, H, W, KH, KW, PAD):
    """Per-core input map from one batch slice (numpy, f32)."""
    C = x_b.shape[0]
    Cout = weight.shape[0]
    HW = H * W
    S = HW // P
    NT = KH * KW
    CB = C // P
    NSTRIPE = 8
    SPX = HW // NSTRIPE
    NIDX = 4 * SPX
    SW = NIDX // 16

    xt = np.ascontiguousarray(x_b.reshape(C, HW).T).astype(np_bf16)
    off = offset_b.reshape(NT, 2, HW)
    j = np.arange(HW)
    ks = np.arange(NT)
    byv = (j[None, :] // W - PAD + (ks // KW)[:, None]).astype(np.float32)  # [k, j]
    bxv = (j[None, :] % W - PAD + (ks % KW)[:, None]).astype(np.float32)

    def l128(a):  # [k, j] -> [p, k, s], j = 128*s + p
        return np.ascontiguousarray(a.reshape(NT, S, P).transpose(2, 0, 1)).astype(np.float32)

    # per-corner clamped gather rows, mirroring the device f32 floor trick
    py = off[:, 0].astype(np.float32) + byv
    px = off[:, 1].astype(np.float32) + bxv
    FADD = np.float32(4.0 * max(H, W) - 0.5)
    y0 = np.rint(py + FADD).astype(np.int64) - int(4.0 * max(H, W))
    x0 = np.rint(px + FADD).astype(np.int64) - int(4.0 * max(H, W))
    rq = np.zeros((4, NT, HW), np.int64)
    for n in range(4):
        yn = np.clip(y0 + (n >> 1), 0, H - 1)
        xn = np.clip(x0 + (n & 1), 0, W - 1)
        rq[n] = yn * W + xn

    # slot order per (stripe, k): i = 128*(4*sl+d) + 32*n + q,
    # pixel = stripe*SPX + 128*sl + 32*d + q
    i_arr = np.arange(NIDX)
    sl_i = i_arr // 512
    d_i = (i_arr // 128) % 4
    n_i = (i_arr % 128) // 32
    q_i = i_arr % 32
    pxl = 128 * sl_i + 32 * d_i + q_i      # [NIDX]
    idx = np.zeros((NSTRIPE, P, NT, SW), np.int16)
    for st in range(NSTRIPE):
        rows = rq[n_i, :, st * SPX + pxl].astype(np.int16)  # [NIDX, NT]
        wrap = rows.reshape(SW, 16, NT).transpose(1, 2, 0)  # [16, NT, SW]
        idx[st] = np.tile(wrap, (8, 1, 1))                  # [128, NT, SW]

    wr = weight.reshape(Cout, C, NT)
    wtv = wr.reshape(Cout, CB, P, NT).transpose(2, 3, 1, 0)  # [p, k, cb, o]
    kmat = (np.arange(P)[:, None] % 32 == np.arange(32)[None, :])
    # permutation constants: E[n,d][p,i] = 1 iff p == 32d+q and i == 32n+q
    E = np.zeros((4, 4, P, P), np.float32)
    q = np.arange(32)
    for n in range(4):
        for d in range(4):
            E[n, d, 32 * d + q, 32 * n + q] = 1.0
    permv = np.ascontiguousarray(E.transpose(2, 0, 1, 3))  # [p, n, d, i]

    return {
        "xtab": xt,
        "idxp": idx,
        "offs": np.stack([l128(off[:, 0]), l128(off[:, 1]),
                          l128(byv), l128(bxv)], axis=1),
        "wt": np.ascontiguousarray(wtv).astype(np_bf16),
        "kmat": kmat.astype(np_bf16),
        "perm": permv.astype(np_bf16),
    }


_NC_CACHE = {}


def _get_nc(key, **kw):
    if key not in _NC_CACHE:
        _NC_CACHE[key] = build_dcn(**kw)
    return _NC_CACHE[key]


def kernel(x, offset, weight):
    x = np.asarray(x, dtype=np.float32)
    offset = np.asarray(offset, dtype=np.float32)
    weight = np.asarray(weight, dtype=np.float32)
    B, C, H, W = x.shape
    Cout = weight.shape[0]
    KH, KW = weight.shape[2], weight.shape[3]
    PAD = 1
    assert B == 8 and C % 128 == 0 and Cout % 128 == 0
    nc = _get_nc((C, Cout, H, W, KH, KW), C=C, Cout=Cout, H=H, W=W,
                 NT=KH * KW)
    in_maps = [host_prep(x[b], offset[b], weight, H, W, KH, KW, PAD)
               for b in range(B)]
    res = run_bass_kernel_spmd(nc, in_maps, list(range(B)))
    out = np.stack([res.results[b]["out"].astype(np.float32).reshape(Cout, H, W)
                    for b in range(B)])
    return out
